# revision 4
# baseline (speedup 1.0000x reference)
"""Causal self-attention (B=2, T=2048, C=1024, H=16) on 8 TRN2 NeuronCores.

Sharding: core = b * 4 + g  (b in 0..1 batches, g in 0..3 head-groups of 4 heads).
Each core computes qkv projection for its 4 heads, causal flash-style attention,
and the output projection restricted to its heads' rows of w_proj, producing a
partial y[b] (bf16). Host sums the 4 partials per batch and folds in the exact
bias terms (b_qk applied on device; b_v and b_proj folded algebraically on host:
att rows sum to 1 so  att@(v + 1 b_v^T) @ w_p = att@v@w_p + b_v@w_p).

Device compute is bf16 matmuls with fp32 PSUM accumulation:
 - q^T/k^T in head-pair-stacked layout [128, T]; v in [T, 4*(64+1)] layout with
   a ones column appended per head, so each AV matmul (lhsT [128,65]) also
   accumulates the softmax denominator l into row 64 of that head's O^T psum.
 - S^T blocks via K=64 matmul pairs; exp on ACT with exact causal column
   regions; triangular-mask mul on DVE for diagonal blocks.
 - per-head normalization: 1/l (DVE) -> K=1 matmul broadcast -> DVE mul.
 - projection back to natural [T, C] layout (lhsT = O^T blocks), DMA out bf16.

Pipeline: one software pipeline over 512-token chunks R - chunk R's attention
(ACT-exp paced) is interleaved with chunk R+1's qkv projection and earlier
chunks' output projections as background PE work. PSUM (8 banks): S pair-tiles
[128,1024] x2 bufs (4 banks, also the rotation used by qkv/proj/broadcast
pieces), per-head O^T+l accumulators x4 (4 banks).
"""

import sys

if "/opt/trn_rl_repo" not in sys.path:
    sys.path.insert(0, "/opt/trn_rl_repo")

import numpy as np
import ml_dtypes

BF16 = ml_dtypes.bfloat16
B, T_FULL, C = 2, 2048, 1024
H, HD = 16, 64
HPC = 4  # heads per core
NCORES = 8
NK = C // 128  # contraction k-tiles


def build_nc(T, num_devices=NCORES, interleave=True):
    import concourse.bass as bass
    import concourse.tile as tile
    from concourse import bacc, mybir

    bf = mybir.dt.bfloat16
    f32 = mybir.dt.float32
    NT = T // 128   # token tiles
    NCH = T // 512  # token chunks

    nc = bacc.Bacc("TRN2", target_bir_lowering=False, debug=False,
                   num_devices=num_devices)

    xT_d = nc.dram_tensor("xT", [C, T], bf, kind="ExternalInput")
    wqkv_d = nc.dram_tensor("wqkv", [C, 768], bf, kind="ExternalInput")
    wp_d = nc.dram_tensor("wp", [256, C], bf, kind="ExternalInput")
    bqk_d = nc.dram_tensor("bqk", [128, 4], f32, kind="ExternalInput")
    cst_d = nc.dram_tensor("cst", [128, 132], bf, kind="ExternalInput")
    y_d = nc.dram_tensor("y", [T, C], bf, kind="ExternalOutput")

    Exp = mybir.ActivationFunctionType.Exp
    PSUM = bass.MemorySpace.PSUM

    with tile.TileContext(nc) as tc, nc.allow_low_precision(
            reason="bf16 activations by design; fp32 PSUM accumulation"):
        with (
            tc.tile_pool(name="const", bufs=1) as cpool,
            tc.tile_pool(name="act", bufs=1) as apool,
            tc.tile_pool(name="se", bufs=5) as sepool,
            tc.tile_pool(name="small", bufs=2) as spool,
            tc.tile_pool(name="ysb", bufs=6) as ypool,
        ):
            # ---- constant/weight loads ----
            # order: qkv weights + chunk-0 x first so PE starts early
            xT = [cpool.tile([128, T], bf, tag=f"xT{k}", name=f"xT{k}")
                  for k in range(NK)]
            wqkv = [cpool.tile([128, 768], bf, tag=f"wqkv{k}", name=f"wqkv{k}")
                    for k in range(NK)]
            wqk = [t[:, 0:512] for t in wqkv]
            wv = [t[:, 512:768] for t in wqkv]
            wp = [cpool.tile([128, C], bf, tag=f"wp{k}", name=f"wp{k}")
                  for k in range(2)]
            xh = min(1024, T)
            for k in range(NK):
                nc.sync.dma_start(wqkv[k][:], wqkv_d.ap()[128 * k:128 * (k + 1), :])
                nc.sync.dma_start(
                    xT[k][:, 0:xh], xT_d.ap()[128 * k:128 * (k + 1), 0:xh])
            bqk = cpool.tile([128, 4], f32, tag="bqk", name="bqk")
            nc.sync.dma_start(bqk[:], bqk_d.ap()[:])
            cst = cpool.tile([128, 132], bf, tag="cst", name="cst")
            nc.sync.dma_start(cst[:], cst_d.ap()[:])
            tri = cst[:, 0:128]
            ones4 = cst[:, 128:132]
            if xh < T:
                for k in range(NK):
                    nc.sync.dma_start(
                        xT[k][:, xh:T], xT_d.ap()[128 * k:128 * (k + 1), xh:T])
            for k in range(2):
                nc.sync.dma_start(wp[k][:], wp_d.ap()[128 * k:128 * (k + 1), :])

            # ---- persistent activations ----
            # qk_sb[m]: m=0 q(pair ab), 1 q(pair cd), 2 k(ab), 3 k(cd)
            qk_sb = [apool.tile([128, T], bf, tag=f"qk{m}", name=f"qk{m}") for m in range(4)]
            # v tiles: per head 65 cols = [v_h (64) | ones]; l rides AV matmuls
            v_sb = [apool.tile([128, 260], bf, tag=f"v{t}", name=f"v{t}") for t in range(NT)]
            O_sb = [apool.tile([128, T], bf, tag=f"O{p}", name=f"O{p}") for p in range(2)]
            # ones columns of v tiles (written once; v-piece copies skip them)
            for t in range(NT):
                v3 = v_sb[t].rearrange("p (g c) -> p g c", g=4)
                nc.vector.tensor_copy(v3[:, :, 64:65], ones4[:, :, None])

            # ---- single PSUM layout for all phases (8 banks):
            #   "S" x2 bufs [128,1024] = 4 banks (S blocks + qkv/proj/bcast)
            #   O0..O3 per-head accumulators (rows 0:64 O^T, row 64 l) = 4
            with (
                tc.tile_pool(name="ps_s", bufs=2, space=PSUM) as ps_s,
                tc.tile_pool(name="ps_o", bufs=1, space=PSUM) as ps_o,
            ):
                def emit_qkv_piece(c, piece):
                    """piece 0..3: q/k m-tile; 4..7: v token-tile."""
                    if piece < 4:
                        m = piece
                        pt = ps_s.tile([128, 512], f32, tag="S", name="qkp")
                        for k in range(NK):
                            nc.tensor.matmul(
                                pt[:],
                                wqk[k][:, 128 * m:128 * (m + 1)],
                                xT[k][:, 512 * c:512 * (c + 1)],
                                start=(k == 0), stop=(k == NK - 1),
                            )
                        nc.vector.tensor_scalar_add(
                            qk_sb[m][:, 512 * c:512 * (c + 1)], pt[:],
                            bqk[:, m:m + 1],
                        )
                    else:
                        tt = 4 * c + piece - 4
                        pv = ps_s.tile([128, 256], f32, tag="S", name="vp")
                        for k in range(NK):
                            nc.tensor.matmul(
                                pv[:],
                                xT[k][:, 128 * tt:128 * (tt + 1)],
                                wv[k],
                                start=(k == 0), stop=(k == NK - 1),
                            )
                        v3 = v_sb[tt].rearrange("p (g c) -> p g c", g=4)
                        nc.vector.tensor_copy(
                            v3[:, :, 0:64],
                            pv.rearrange("p (g c) -> p g c", g=4))

                def emit_proj_piece(R, piece):
                    tt = 4 * R + piece // 2
                    cc = piece % 2
                    yp = ps_s.tile([128, 512], f32, tag="S", name="y")
                    for kd in range(2):
                        nc.tensor.matmul(
                            yp[:],
                            O_sb[kd][:, 128 * tt:128 * (tt + 1)],
                            wp[kd][:, 512 * cc:512 * (cc + 1)],
                            start=(kd == 0), stop=(kd == 1),
                        )
                    ysb = ypool.tile([128, 512], bf, tag="ysb", name="ysb")
                    nc.vector.tensor_copy(ysb[:], yp[:])
                    nc.sync.dma_start(
                        y_d.ap()[128 * tt:128 * (tt + 1),
                                 512 * cc:512 * (cc + 1)],
                        ysb[:])

                # qkv for chunk 0 up front; later chunks + projections are
                # interleaved into the attention loop as background pieces to
                # keep PE continuously busy during ACT-paced sections.
                nhead = min(1, NCH)
                for c in range(nhead):
                    for piece in range(8):
                        emit_qkv_piece(c, piece)
                if not interleave:
                    for c in range(nhead, NCH):
                        for piece in range(8):
                            emit_qkv_piece(c, piece)

                bg = []  # deferred (fn, args) pieces
                for R in range(NCH):
                    if interleave and R + nhead < NCH:
                        bg.extend(("qkv", R + nhead, p) for p in range(8))
                    if interleave and NCH == 4:
                        # deadline-based: defer early projections so the
                        # filler-starved late chunks get background PE work
                        if R == 2:
                            bg.extend(("proj", 0, p) for p in range(8))
                        elif R == 3:
                            bg.extend(("proj", rr, p) for rr in (1, 2)
                                      for p in range(8))
                    O_ps = [ps_o.tile([128, 512], f32, tag=f"O{h}",
                                      name=f"O{h}") for h in range(4)]
                    njr = 4 * R + 4
                    nbg0 = len(bg)
                    emitted = 0
                    for j in range(njr):
                        m = j - 4 * R
                        lo = 128 * m if m >= 0 else 0
                        last = (j == njr - 1)
                        st = (j == 0)
                        Ses = []
                        for pi in range(2):
                            qT = qk_sb[pi]
                            kT = qk_sb[2 + pi]
                            Sp = ps_s.tile([128, 1024], f32, tag="S", name="S")
                            # S^T block: heads 2pi (cols 0:512), 2pi+1 (512:)
                            nc.tensor.matmul(
                                Sp[:, lo:512],
                                kT[0:64, 128 * j:128 * (j + 1)],
                                qT[0:64, 512 * R + lo:512 * (R + 1)],
                                start=True, stop=True,
                            )
                            nc.tensor.matmul(
                                Sp[:, 512 + lo:1024],
                                kT[64:128, 128 * j:128 * (j + 1)],
                                qT[64:128, 512 * R + lo:512 * (R + 1)],
                                start=True, stop=True,
                            )
                            Se = sepool.tile([128, 1024], bf, tag="Se", name="Se")
                            sp3 = Sp.rearrange("p (h n) -> p h n", h=2)
                            se3 = Se.rearrange("p (h n) -> p h n", h=2)
                            nc.scalar.activation(
                                se3[:, :, lo:512], sp3[:, :, lo:512], Exp,
                                scale=0.125,
                            )
                            if m >= 0:
                                # diagonal 128-block: upper-tri (incl diag)
                                nc.vector.tensor_mul(
                                    Se[:, lo:lo + 128], Se[:, lo:lo + 128], tri)
                                nc.vector.tensor_mul(
                                    Se[:, 512 + lo:512 + lo + 128],
                                    Se[:, 512 + lo:512 + lo + 128], tri)
                            Ses.append(Se)
                        # AV with fused l row: lhsT = [v_h | ones] (65 cols)
                        for h in range(4):
                            pi, ch = divmod(h, 2)
                            nc.tensor.matmul(
                                O_ps[h][0:65, lo:512],
                                v_sb[j][:, 65 * h:65 * h + 65],
                                Ses[pi][:, 512 * ch + lo:512 * (ch + 1)],
                                start=st, stop=last,
                            )
                        # spread background pieces (next chunk's qkv, earlier
                        # chunks' projections) across the attention loop --
                        # emitted after the j-group so pair matmuls outrank
                        # fillers in scheduler priority
                        want = (nbg0 * (j + 1) + njr - 1) // njr
                        while emitted < want and bg:
                            kind, rr, p = bg.pop(0)
                            emitted += 1
                            if kind == "qkv":
                                emit_qkv_piece(rr, p)
                            else:
                                emit_proj_piece(rr, p)
                    # normalization per head: rl = 1/l on the l row, broadcast
                    # to 64 partitions by a K=1 matmul, then scale O^T
                    for h in range(4):
                        pi, ch = divmod(h, 2)
                        rl = spool.tile([128, 512], bf, tag=f"rl{h}",
                                        name=f"rl{h}")
                        nc.vector.reciprocal(rl[64:65, :], O_ps[h][64:65, :])
                        bcp = ps_s.tile([128, 512], f32, tag="S", name="bc")
                        # K=1 matmul bcast: out[m,n] = 1 * rl[0,n]; lhsT is an
                        # all-ones row of tri at partition 64 (matches rhs base)
                        nc.tensor.matmul(
                            bcp[0:64, :], tri[64:65, 64:128],
                            rl[64:65, :], start=True, stop=True,
                        )
                        bcs = spool.tile([128, 512], bf, tag="bcs", name="bcs")
                        nc.vector.tensor_copy(bcs[0:64, :], bcp[0:64, :])
                        nc.vector.tensor_mul(
                            O_sb[pi][64 * ch:64 * ch + 64,
                                     512 * R:512 * (R + 1)],
                            O_ps[h][0:64, :], bcs[0:64, :])
                    # projection for this chunk rides a later chunk's loop
                    if interleave:
                        if NCH != 4 or R == NCH - 1:
                            bg.extend(("proj", R, p) for p in range(8))
                    else:
                        for p in range(8):
                            emit_proj_piece(R, p)

                # flush remaining pieces
                for kind, rr, p in bg:
                    if kind == "qkv":
                        emit_qkv_piece(rr, p)
                    else:
                        emit_proj_piece(rr, p)

    nc.compile()
    return nc


def make_core_inputs(x, w_qkv, b_qkv, w_proj, core, T=None):
    """Host-side shard/prep for one core. Returns the in_map dict."""
    if T is None:
        T = x.shape[1]
    b, g = divmod(core, 4)
    heads = [4 * g + i for i in range(HPC)]

    xT = np.ascontiguousarray(np.asarray(x[b], np.float32).T).astype(BF16)

    qcols = [w_qkv[:, h * HD:(h + 1) * HD] for h in heads]
    kcols = [w_qkv[:, C + h * HD:C + (h + 1) * HD] for h in heads]
    vcols = [w_qkv[:, 2 * C + h * HD:2 * C + (h + 1) * HD] for h in heads]
    wqk = np.concatenate(qcols + kcols, axis=1).astype(BF16)      # [C, 512]
    wv = np.concatenate(vcols, axis=1).astype(BF16)               # [C, 256]
    wp = np.concatenate([w_proj[h * HD:(h + 1) * HD, :] for h in heads],
                        axis=0).astype(BF16)                      # [256, C]

    bq = [b_qkv[h * HD:(h + 1) * HD] for h in heads]
    bk = [b_qkv[C + h * HD:C + (h + 1) * HD] for h in heads]
    bqk = np.concatenate(bq + bk).astype(np.float32).reshape(4, 128).T
    bqk = np.ascontiguousarray(bqk)                               # [128, 4]

    a = np.arange(128)
    tri = (a[:, None] <= a[None, :]).astype(BF16)                 # [128, 128]
    ones4 = np.ones((128, 4), dtype=BF16)

    return {
        "xT": xT, "wqkv": np.concatenate([wqk, wv], axis=1),
        "wp": wp, "bqk": bqk,
        "cst": np.concatenate([tri, ones4], axis=1),
    }


_compiled = {}


def _get_nc(T):
    if T not in _compiled:
        _compiled[T] = build_nc(T)
    return _compiled[T]


def kernel(x, w_qkv, b_qkv, w_proj, b_proj):
    from concourse.bass_utils import run_bass_kernel_spmd

    x = np.asarray(x, np.float32)
    w_qkv = np.asarray(w_qkv, np.float32)
    b_qkv = np.asarray(b_qkv, np.float32)
    w_proj = np.asarray(w_proj, np.float32)
    b_proj = np.asarray(b_proj, np.float32)
    T = x.shape[1]

    nc = _get_nc(T)
    in_maps = [make_core_inputs(x, w_qkv, b_qkv, w_proj, core, T)
               for core in range(NCORES)]
    res = run_bass_kernel_spmd(nc, in_maps, core_ids=list(range(NCORES)))

    y = np.zeros((B, T, C), np.float32)
    for core in range(NCORES):
        b = core // 4
        y[b] += res.results[core]["y"].astype(np.float32)
    y += b_proj[None, None, :] + (b_qkv[2 * C:3 * C] @ w_proj)[None, None, :]
    return y


# revision 6
# speedup vs baseline: 1.0697x; 1.0697x over previous
"""Causal self-attention (B=2, T=2048, C=1024, H=16) on 8 TRN2 NeuronCores.

Sharding: core = b * 4 + g  (b in 0..1 batches, g in 0..3 head-groups of 4 heads).
Each core computes qkv projection for its 4 heads, causal flash-style attention,
and the output projection restricted to its heads' rows of w_proj, producing a
partial y[b] (bf16). Host sums the 4 partials per batch and folds in the exact
bias terms (b_qk applied on device; b_v and b_proj folded algebraically on host:
att rows sum to 1 so  att@(v + 1 b_v^T) @ w_p = att@v@w_p + b_v@w_p).

Device compute is bf16 matmuls with fp32 PSUM accumulation:
 - q^T/k^T in head-pair-stacked layout [128, T]; v in [T, 4*(64+1)] layout with
   a ones column appended per head, so each AV matmul (lhsT [128,65]) also
   accumulates the softmax denominator l into row 64 of that head's O^T psum.
 - S^T blocks via K=64 matmul pairs; exp on ACT with exact causal column
   regions; triangular-mask mul on DVE for diagonal blocks.
 - per-head normalization: 1/l (DVE) -> K=1 matmul broadcast -> DVE mul.
 - projection back to natural [T, C] layout (lhsT = O^T blocks), DMA out bf16.

Pipeline: one software pipeline over 512-token chunks R - chunk R's attention
(ACT-exp paced) is interleaved with chunk R+1's qkv projection and earlier
chunks' output projections as background PE work. PSUM (8 banks): S pair-tiles
[128,1024] x2 bufs (4 banks, also the rotation used by qkv/proj/broadcast
pieces), per-head O^T+l accumulators x4 (4 banks).
"""

import sys

if "/opt/trn_rl_repo" not in sys.path:
    sys.path.insert(0, "/opt/trn_rl_repo")

import numpy as np
import ml_dtypes

BF16 = ml_dtypes.bfloat16
B, T_FULL, C = 2, 2048, 1024
H, HD = 16, 64
HPC = 4  # heads per core
NCORES = 8
NK = C // 128  # contraction k-tiles


def build_nc(T, num_devices=NCORES, interleave=True):
    import concourse.bass as bass
    import concourse.tile as tile
    from concourse import bacc, mybir

    bf = mybir.dt.bfloat16
    f32 = mybir.dt.float32
    NT = T // 128   # token tiles
    NCH = T // 512  # token chunks

    nc = bacc.Bacc("TRN2", target_bir_lowering=False, debug=False,
                   num_devices=num_devices)

    xT_d = nc.dram_tensor("xT", [C, T], bf, kind="ExternalInput")
    wqkv_d = nc.dram_tensor("wqkv", [C, 768], bf, kind="ExternalInput")
    wp_d = nc.dram_tensor("wp", [256, C], bf, kind="ExternalInput")
    bqk_d = nc.dram_tensor("bqk", [128, 4], f32, kind="ExternalInput")
    cst_d = nc.dram_tensor("cst", [128, 192], bf, kind="ExternalInput")
    y_d = nc.dram_tensor("y", [T, C], bf, kind="ExternalOutput")

    Exp = mybir.ActivationFunctionType.Exp
    PSUM = bass.MemorySpace.PSUM

    with tile.TileContext(nc) as tc, nc.allow_low_precision(
            reason="bf16 activations by design; fp32 PSUM accumulation"):
        with (
            tc.tile_pool(name="const", bufs=1) as cpool,
            tc.tile_pool(name="act", bufs=1) as apool,
            tc.tile_pool(name="se", bufs=5) as sepool,
            tc.tile_pool(name="small", bufs=2) as spool,
            tc.tile_pool(name="ysb", bufs=6) as ypool,
        ):
            # ---- constant/weight loads ----
            # order: qkv weights + chunk-0 x first so PE starts early
            xT = [cpool.tile([128, T], bf, tag=f"xT{k}", name=f"xT{k}")
                  for k in range(NK)]
            wqkv = [cpool.tile([128, 768], bf, tag=f"wqkv{k}", name=f"wqkv{k}")
                    for k in range(NK)]
            wqk = [t[:, 0:512] for t in wqkv]
            wv = [t[:, 512:768] for t in wqkv]
            wp = [cpool.tile([128, C], bf, tag=f"wp{k}", name=f"wp{k}")
                  for k in range(2)]
            xh = min(1024, T)
            for k in range(NK):
                nc.sync.dma_start(wqkv[k][:], wqkv_d.ap()[128 * k:128 * (k + 1), :])
                nc.sync.dma_start(
                    xT[k][:, 0:xh], xT_d.ap()[128 * k:128 * (k + 1), 0:xh])
            bqk = cpool.tile([128, 4], f32, tag="bqk", name="bqk")
            nc.sync.dma_start(bqk[:], bqk_d.ap()[:])
            cst = cpool.tile([128, 192], bf, tag="cst", name="cst")
            nc.sync.dma_start(cst[:], cst_d.ap()[:])
            tri = cst[:, 0:128]
            ones64 = cst[:, 128:192]
            if xh < T:
                for k in range(NK):
                    nc.sync.dma_start(
                        xT[k][:, xh:T], xT_d.ap()[128 * k:128 * (k + 1), xh:T])
            for k in range(2):
                nc.sync.dma_start(wp[k][:], wp_d.ap()[128 * k:128 * (k + 1), :])

            # ---- persistent activations ----
            # qk_sb[m]: m=0 q(pair ab), 1 q(pair cd), 2 k(ab), 3 k(cd)
            qk_sb = [apool.tile([128, T], bf, tag=f"qk{m}", name=f"qk{m}") for m in range(4)]
            # v tiles: per head 128 cols = [v_h(64) | ones(64)]; the AV matmul
            # then yields O^T in psum rows 0:64 and l replicated in rows 64:128
            v_sb = [apool.tile([128, 512], bf, tag=f"v{t}", name=f"v{t}") for t in range(NT)]
            O_sb = [apool.tile([128, T], bf, tag=f"O{p}", name=f"O{p}") for p in range(2)]
            # ones blocks of v tiles (written once; v-piece copies skip them)
            for t in range(NT):
                v3 = v_sb[t].rearrange("p (g c) -> p g c", g=4)
                for h in range(4):
                    nc.vector.tensor_copy(v3[:, h, 64:128], ones64)

            # ---- single PSUM layout for all phases (8 banks):
            #   "S" x2 bufs [128,1024] = 4 banks (S blocks + qkv/proj/bcast)
            #   O0..O3 per-head accumulators (rows 0:64 O^T, row 64 l) = 4
            with (
                tc.tile_pool(name="ps_s", bufs=2, space=PSUM) as ps_s,
                tc.tile_pool(name="ps_o", bufs=1, space=PSUM) as ps_o,
            ):
                def emit_qkv_piece(c, piece):
                    """piece 0..3: q/k m-tile; 4..7: v token-tile."""
                    if piece < 4:
                        m = piece
                        pt = ps_s.tile([128, 512], f32, tag="S", name="qkp")
                        for k in range(NK):
                            nc.tensor.matmul(
                                pt[:],
                                wqk[k][:, 128 * m:128 * (m + 1)],
                                xT[k][:, 512 * c:512 * (c + 1)],
                                start=(k == 0), stop=(k == NK - 1),
                            )
                        nc.vector.tensor_scalar_add(
                            qk_sb[m][:, 512 * c:512 * (c + 1)], pt[:],
                            bqk[:, m:m + 1],
                        )
                    else:
                        tt = 4 * c + piece - 4
                        pv = ps_s.tile([128, 256], f32, tag="S", name="vp")
                        for k in range(NK):
                            nc.tensor.matmul(
                                pv[:],
                                xT[k][:, 128 * tt:128 * (tt + 1)],
                                wv[k],
                                start=(k == 0), stop=(k == NK - 1),
                            )
                        v3 = v_sb[tt].rearrange("p (g c) -> p g c", g=4)
                        nc.vector.tensor_copy(
                            v3[:, :, 0:64],
                            pv.rearrange("p (g c) -> p g c", g=4))

                def emit_proj_piece(R, piece):
                    tt = 4 * R + piece // 2
                    cc = piece % 2
                    yp = ps_s.tile([128, 512], f32, tag="S", name="y")
                    for kd in range(2):
                        nc.tensor.matmul(
                            yp[:],
                            O_sb[kd][:, 128 * tt:128 * (tt + 1)],
                            wp[kd][:, 512 * cc:512 * (cc + 1)],
                            start=(kd == 0), stop=(kd == 1),
                        )
                    ysb = ypool.tile([128, 512], bf, tag="ysb", name="ysb")
                    nc.vector.tensor_copy(ysb[:], yp[:])
                    nc.sync.dma_start(
                        y_d.ap()[128 * tt:128 * (tt + 1),
                                 512 * cc:512 * (cc + 1)],
                        ysb[:])

                # qkv for chunk 0 up front; later chunks + projections are
                # interleaved into the attention loop as background pieces to
                # keep PE continuously busy during ACT-paced sections.
                nhead = min(1, NCH)
                for c in range(nhead):
                    for piece in range(8):
                        emit_qkv_piece(c, piece)
                if not interleave:
                    for c in range(nhead, NCH):
                        for piece in range(8):
                            emit_qkv_piece(c, piece)

                bg = []  # deferred (fn, args) pieces
                for R in range(NCH):
                    if interleave and R + nhead < NCH:
                        bg.extend(("qkv", R + nhead, p) for p in range(8))
                    if interleave and NCH == 4:
                        # deadline-based: defer early projections so the
                        # filler-starved late chunks get background PE work
                        if R == 2:
                            bg.extend(("proj", 0, p) for p in range(8))
                        elif R == 3:
                            bg.extend(("proj", rr, p) for rr in (1, 2)
                                      for p in range(8))
                    O_ps = [ps_o.tile([128, 512], f32, tag=f"O{h}",
                                      name=f"O{h}") for h in range(4)]
                    njr = 4 * R + 4
                    nbg0 = len(bg)
                    emitted = 0
                    for j in range(njr):
                        m = j - 4 * R
                        lo = 128 * m if m >= 0 else 0
                        last = (j == njr - 1)
                        st = (j == 0)
                        Ses = []
                        for pi in range(2):
                            qT = qk_sb[pi]
                            kT = qk_sb[2 + pi]
                            Sp = ps_s.tile([128, 1024], f32, tag="S", name="S")
                            # S^T block: heads 2pi (cols 0:512), 2pi+1 (512:)
                            nc.tensor.matmul(
                                Sp[:, lo:512],
                                kT[0:64, 128 * j:128 * (j + 1)],
                                qT[0:64, 512 * R + lo:512 * (R + 1)],
                                start=True, stop=True,
                            )
                            nc.tensor.matmul(
                                Sp[:, 512 + lo:1024],
                                kT[64:128, 128 * j:128 * (j + 1)],
                                qT[64:128, 512 * R + lo:512 * (R + 1)],
                                start=True, stop=True,
                            )
                            Se = sepool.tile([128, 1024], bf, tag="Se", name="Se")
                            sp3 = Sp.rearrange("p (h n) -> p h n", h=2)
                            se3 = Se.rearrange("p (h n) -> p h n", h=2)
                            nc.scalar.activation(
                                se3[:, :, lo:512], sp3[:, :, lo:512], Exp,
                                scale=0.125,
                            )
                            if m >= 0:
                                # diagonal 128-block: upper-tri (incl diag)
                                nc.vector.tensor_mul(
                                    Se[:, lo:lo + 128], Se[:, lo:lo + 128], tri)
                                nc.vector.tensor_mul(
                                    Se[:, 512 + lo:512 + lo + 128],
                                    Se[:, 512 + lo:512 + lo + 128], tri)
                            Ses.append(Se)
                        # AV with fused l: lhsT = [v_h | ones64] (128 cols) ->
                        # rows 0:64 = O^T_h, rows 64:128 = l replicated
                        for h in range(4):
                            pi, ch = divmod(h, 2)
                            nc.tensor.matmul(
                                O_ps[h][:, lo:512],
                                v_sb[j][:, 128 * h:128 * (h + 1)],
                                Ses[pi][:, 512 * ch + lo:512 * (ch + 1)],
                                start=st, stop=last,
                            )
                        # spread background pieces (next chunk's qkv, earlier
                        # chunks' projections) across the attention loop --
                        # emitted after the j-group so pair matmuls outrank
                        # fillers in scheduler priority
                        want = (nbg0 * (j + 1) + njr - 1) // njr
                        while emitted < want and bg:
                            kind, rr, p = bg.pop(0)
                            emitted += 1
                            if kind == "qkv":
                                emit_qkv_piece(rr, p)
                            else:
                                emit_proj_piece(rr, p)
                    # normalization per head, pure DVE: 1/l on the replicated
                    # l rows 64:128, then partition-shifted multiply into O_sb
                    for h in range(4):
                        pi, ch = divmod(h, 2)
                        rl = spool.tile([128, 512], bf, tag=f"rl{h}",
                                        name=f"rl{h}")
                        nc.vector.reciprocal(rl[64:128, :], O_ps[h][64:128, :])
                        nc.vector.tensor_mul(
                            O_sb[pi][64 * ch:64 * ch + 64,
                                     512 * R:512 * (R + 1)],
                            O_ps[h][0:64, :], rl[64:128, :])
                    # projection for this chunk rides a later chunk's loop
                    if interleave:
                        if NCH != 4 or R == NCH - 1:
                            bg.extend(("proj", R, p) for p in range(8))
                    else:
                        for p in range(8):
                            emit_proj_piece(R, p)

                # flush remaining pieces
                for kind, rr, p in bg:
                    if kind == "qkv":
                        emit_qkv_piece(rr, p)
                    else:
                        emit_proj_piece(rr, p)

    nc.compile()
    return nc


def make_core_inputs(x, w_qkv, b_qkv, w_proj, core, T=None):
    """Host-side shard/prep for one core. Returns the in_map dict."""
    if T is None:
        T = x.shape[1]
    b, g = divmod(core, 4)
    heads = [4 * g + i for i in range(HPC)]

    xT = np.ascontiguousarray(np.asarray(x[b], np.float32).T).astype(BF16)

    qcols = [w_qkv[:, h * HD:(h + 1) * HD] for h in heads]
    kcols = [w_qkv[:, C + h * HD:C + (h + 1) * HD] for h in heads]
    vcols = [w_qkv[:, 2 * C + h * HD:2 * C + (h + 1) * HD] for h in heads]
    wqk = np.concatenate(qcols + kcols, axis=1).astype(BF16)      # [C, 512]
    wv = np.concatenate(vcols, axis=1).astype(BF16)               # [C, 256]
    wp = np.concatenate([w_proj[h * HD:(h + 1) * HD, :] for h in heads],
                        axis=0).astype(BF16)                      # [256, C]

    bq = [b_qkv[h * HD:(h + 1) * HD] for h in heads]
    bk = [b_qkv[C + h * HD:C + (h + 1) * HD] for h in heads]
    bqk = np.concatenate(bq + bk).astype(np.float32).reshape(4, 128).T
    bqk = np.ascontiguousarray(bqk)                               # [128, 4]

    a = np.arange(128)
    tri = (a[:, None] <= a[None, :]).astype(BF16)                 # [128, 128]
    ones64 = np.ones((128, 64), dtype=BF16)

    return {
        "xT": xT, "wqkv": np.concatenate([wqk, wv], axis=1),
        "wp": wp, "bqk": bqk,
        "cst": np.concatenate([tri, ones64], axis=1),
    }


_compiled = {}


def _get_nc(T):
    if T not in _compiled:
        _compiled[T] = build_nc(T)
    return _compiled[T]


def kernel(x, w_qkv, b_qkv, w_proj, b_proj):
    from concourse.bass_utils import run_bass_kernel_spmd

    x = np.asarray(x, np.float32)
    w_qkv = np.asarray(w_qkv, np.float32)
    b_qkv = np.asarray(b_qkv, np.float32)
    w_proj = np.asarray(w_proj, np.float32)
    b_proj = np.asarray(b_proj, np.float32)
    T = x.shape[1]

    nc = _get_nc(T)
    in_maps = [make_core_inputs(x, w_qkv, b_qkv, w_proj, core, T)
               for core in range(NCORES)]
    res = run_bass_kernel_spmd(nc, in_maps, core_ids=list(range(NCORES)))

    y = np.zeros((B, T, C), np.float32)
    for core in range(NCORES):
        b = core // 4
        y[b] += res.results[core]["y"].astype(np.float32)
    y += b_proj[None, None, :] + (b_qkv[2 * C:3 * C] @ w_proj)[None, None, :]
    return y


# revision 8
# speedup vs baseline: 1.1213x; 1.0482x over previous
"""Causal self-attention (B=2, T=2048, C=1024, H=16) on 8 TRN2 NeuronCores.

Sharding: core = b * 4 + g  (b in 0..1 batches, g in 0..3 head-groups of 4 heads).
Each core computes qkv projection for its 4 heads, causal flash-style attention,
and the output projection restricted to its heads' rows of w_proj, producing a
partial y[b] (bf16). Host sums the 4 partials per batch and folds in the exact
bias terms (b_qk applied on device; b_v and b_proj folded algebraically on host:
att rows sum to 1 so  att@(v + 1 b_v^T) @ w_p = att@v@w_p + b_v@w_p).

Device compute is bf16 matmuls with fp32 PSUM accumulation:
 - q^T/k^T in head-pair-stacked layout [128, T]; v in [T, 4*(64+1)] layout with
   a ones column appended per head, so each AV matmul (lhsT [128,65]) also
   accumulates the softmax denominator l into row 64 of that head's O^T psum.
 - S^T blocks via K=64 matmul pairs; exp on ACT with exact causal column
   regions; triangular-mask mul on DVE for diagonal blocks.
 - per-head normalization: 1/l (DVE) -> K=1 matmul broadcast -> DVE mul.
 - projection back to natural [T, C] layout (lhsT = O^T blocks), DMA out bf16.

Pipeline: one software pipeline over 512-token chunks R - chunk R's attention
(ACT-exp paced) is interleaved with chunk R+1's qkv projection and earlier
chunks' output projections as background PE work. PSUM (8 banks): S pair-tiles
[128,1024] x2 bufs (4 banks, also the rotation used by qkv/proj/broadcast
pieces), per-head O^T+l accumulators x4 (4 banks).
"""

import sys

if "/opt/trn_rl_repo" not in sys.path:
    sys.path.insert(0, "/opt/trn_rl_repo")

import numpy as np
import ml_dtypes

BF16 = ml_dtypes.bfloat16
B, T_FULL, C = 2, 2048, 1024
H, HD = 16, 64
HPC = 4  # heads per core
NCORES = 8
NK = C // 128  # contraction k-tiles


def build_nc(T, num_devices=NCORES, interleave=True):
    import concourse.bass as bass
    import concourse.tile as tile
    from concourse import bacc, mybir

    bf = mybir.dt.bfloat16
    f32 = mybir.dt.float32
    NT = T // 128   # token tiles
    NCH = T // 512  # token chunks

    nc = bacc.Bacc("TRN2", target_bir_lowering=False, debug=False,
                   num_devices=num_devices)

    xT_d = nc.dram_tensor("xT", [C, T], bf, kind="ExternalInput")
    wqkv_d = nc.dram_tensor("wqkv", [C, 768], bf, kind="ExternalInput")
    wp_d = nc.dram_tensor("wp", [256, C], bf, kind="ExternalInput")
    bqk_d = nc.dram_tensor("bqk", [128, 4], f32, kind="ExternalInput")
    cst_d = nc.dram_tensor("cst", [128, 192], bf, kind="ExternalInput")
    y_d = nc.dram_tensor("y", [T, C], bf, kind="ExternalOutput")

    Exp = mybir.ActivationFunctionType.Exp
    PSUM = bass.MemorySpace.PSUM

    with tile.TileContext(nc) as tc, nc.allow_low_precision(
            reason="bf16 activations by design; fp32 PSUM accumulation"):
        with (
            tc.tile_pool(name="const", bufs=1) as cpool,
            tc.tile_pool(name="act", bufs=1) as apool,
            tc.tile_pool(name="se", bufs=5) as sepool,
            tc.tile_pool(name="small", bufs=2) as spool,
            tc.tile_pool(name="ysb", bufs=6) as ypool,
        ):
            # ---- constant/weight loads ----
            # order: qkv weights + chunk-0 x first so PE starts early
            xT = [cpool.tile([128, T], bf, tag=f"xT{k}", name=f"xT{k}")
                  for k in range(NK)]
            wqkv = [cpool.tile([128, 768], bf, tag=f"wqkv{k}", name=f"wqkv{k}")
                    for k in range(NK)]
            wqk = [t[:, 0:512] for t in wqkv]
            wv = [t[:, 512:768] for t in wqkv]
            wp = [cpool.tile([128, C], bf, tag=f"wp{k}", name=f"wp{k}")
                  for k in range(2)]
            xh = min(1024, T)
            for k in range(NK):
                nc.sync.dma_start(wqkv[k][:], wqkv_d.ap()[128 * k:128 * (k + 1), :])
                nc.sync.dma_start(
                    xT[k][:, 0:xh], xT_d.ap()[128 * k:128 * (k + 1), 0:xh])
            bqk = cpool.tile([128, 4], f32, tag="bqk", name="bqk")
            nc.sync.dma_start(bqk[:], bqk_d.ap()[:])
            cst = cpool.tile([128, 192], bf, tag="cst", name="cst")
            nc.sync.dma_start(cst[:], cst_d.ap()[:])
            tri = cst[:, 0:128]
            ones64 = cst[:, 128:192]
            if xh < T:
                for k in range(NK):
                    nc.sync.dma_start(
                        xT[k][:, xh:T], xT_d.ap()[128 * k:128 * (k + 1), xh:T])
            for k in range(2):
                nc.sync.dma_start(wp[k][:], wp_d.ap()[128 * k:128 * (k + 1), :])

            # ---- persistent activations ----
            # qk_sb[m]: m=0 q(pair ab), 1 q(pair cd), 2 k(ab), 3 k(cd)
            qk_sb = [apool.tile([128, T], bf, tag=f"qk{m}", name=f"qk{m}") for m in range(4)]
            # v tiles: per head 128 cols = [v_h(64) | ones(64)]; the AV matmul
            # then yields O^T in psum rows 0:64 and l replicated in rows 64:128
            v_sb = [apool.tile([128, 512], bf, tag=f"v{t}", name=f"v{t}") for t in range(NT)]
            O_sb = [apool.tile([128, T], bf, tag=f"O{p}", name=f"O{p}") for p in range(2)]


            # ---- single PSUM layout for all phases (8 banks):
            #   "S" x2 bufs [128,1024] = 4 banks (S blocks + qkv/proj/bcast)
            #   O0..O3 per-head accumulators (rows 0:64 O^T, row 64 l) = 4
            with (
                tc.tile_pool(name="ps_s", bufs=2, space=PSUM) as ps_s,
                tc.tile_pool(name="ps_o", bufs=1, space=PSUM) as ps_o,
            ):
                def emit_qkv_piece(c, piece):
                    """piece 0..3: q/k m-tile; 4..7: v token-tile."""
                    if piece < 4:
                        m = piece
                        pt = ps_s.tile([128, 512], f32, tag="S", name="qkp")
                        for k in range(NK):
                            nc.tensor.matmul(
                                pt[:],
                                wqk[k][:, 128 * m:128 * (m + 1)],
                                xT[k][:, 512 * c:512 * (c + 1)],
                                start=(k == 0), stop=(k == NK - 1),
                            )
                        nc.vector.tensor_scalar_add(
                            qk_sb[m][:, 512 * c:512 * (c + 1)], pt[:],
                            bqk[:, m:m + 1],
                        )
                    else:
                        tt = 4 * c + piece - 4
                        pv = ps_s.tile([128, 256], f32, tag="S", name="vp")
                        for k in range(NK):
                            nc.tensor.matmul(
                                pv[:],
                                xT[k][:, 128 * tt:128 * (tt + 1)],
                                wv[k],
                                start=(k == 0), stop=(k == NK - 1),
                            )
                        v3 = v_sb[tt].rearrange("p (g c) -> p g c", g=4)
                        nc.vector.tensor_copy(
                            v3[:, :, 0:64],
                            pv.rearrange("p (g c) -> p g c", g=4))
                        # ones blocks (l rows source) for this tile, once
                        for h in range(4):
                            nc.vector.tensor_copy(v3[:, h, 64:128], ones64)

                def emit_proj_piece(R, piece):
                    tt = 4 * R + piece // 2
                    cc = piece % 2
                    yp = ps_s.tile([128, 512], f32, tag="S", name="y")
                    for kd in range(2):
                        nc.tensor.matmul(
                            yp[:],
                            O_sb[kd][:, 128 * tt:128 * (tt + 1)],
                            wp[kd][:, 512 * cc:512 * (cc + 1)],
                            start=(kd == 0), stop=(kd == 1),
                        )
                    ysb = ypool.tile([128, 512], bf, tag="ysb", name="ysb")
                    nc.vector.tensor_copy(ysb[:], yp[:])
                    nc.sync.dma_start(
                        y_d.ap()[128 * tt:128 * (tt + 1),
                                 512 * cc:512 * (cc + 1)],
                        ysb[:])

                # qkv for chunk 0 up front; later chunks + projections are
                # interleaved into the attention loop as background pieces to
                # keep PE continuously busy during ACT-paced sections.
                nhead = min(1, NCH)
                for c in range(nhead):
                    for piece in range(8):
                        emit_qkv_piece(c, piece)
                if not interleave:
                    for c in range(nhead, NCH):
                        for piece in range(8):
                            emit_qkv_piece(c, piece)

                bg = []  # deferred (fn, args) pieces
                for R in range(NCH):
                    if interleave and R + nhead < NCH:
                        bg.extend(("qkv", R + nhead, p) for p in range(8))
                    if interleave and NCH == 4:
                        # deadline-based: defer early projections so the
                        # filler-starved late chunks get background PE work
                        if R == 2:
                            bg.extend(("proj", 0, p) for p in range(8))
                        elif R == 3:
                            bg.extend(("proj", rr, p) for rr in (1, 2)
                                      for p in range(8))
                    O_ps = [ps_o.tile([128, 512], f32, tag=f"O{h}",
                                      name=f"O{h}") for h in range(4)]
                    njr = 4 * R + 4
                    nbg0 = len(bg)
                    emitted = 0
                    for j in range(njr):
                        m = j - 4 * R
                        lo = 128 * m if m >= 0 else 0
                        last = (j == njr - 1)
                        st = (j == 0)
                        Ses = []
                        for pi in range(2):
                            qT = qk_sb[pi]
                            kT = qk_sb[2 + pi]
                            Sp = ps_s.tile([128, 1024], f32, tag="S", name="S")
                            # S^T block: heads 2pi (cols 0:512), 2pi+1 (512:)
                            nc.tensor.matmul(
                                Sp[:, lo:512],
                                kT[0:64, 128 * j:128 * (j + 1)],
                                qT[0:64, 512 * R + lo:512 * (R + 1)],
                                start=True, stop=True,
                            )
                            nc.tensor.matmul(
                                Sp[:, 512 + lo:1024],
                                kT[64:128, 128 * j:128 * (j + 1)],
                                qT[64:128, 512 * R + lo:512 * (R + 1)],
                                start=True, stop=True,
                            )
                            Se = sepool.tile([128, 1024], bf, tag="Se", name="Se")
                            sp3 = Sp.rearrange("p (h n) -> p h n", h=2)
                            se3 = Se.rearrange("p (h n) -> p h n", h=2)
                            nc.scalar.activation(
                                se3[:, :, lo:512], sp3[:, :, lo:512], Exp,
                                scale=0.125,
                            )
                            if m >= 0:
                                # diagonal 128-block: upper-tri (incl diag)
                                nc.vector.tensor_mul(
                                    Se[:, lo:lo + 128], Se[:, lo:lo + 128], tri)
                                nc.vector.tensor_mul(
                                    Se[:, 512 + lo:512 + lo + 128],
                                    Se[:, 512 + lo:512 + lo + 128], tri)
                            Ses.append(Se)
                        # AV with fused l: lhsT = [v_h | ones64] (128 cols) ->
                        # rows 0:64 = O^T_h, rows 64:128 = l replicated
                        for h in range(4):
                            pi, ch = divmod(h, 2)
                            nc.tensor.matmul(
                                O_ps[h][:, lo:512],
                                v_sb[j][:, 128 * h:128 * (h + 1)],
                                Ses[pi][:, 512 * ch + lo:512 * (ch + 1)],
                                start=st, stop=last,
                            )
                        # spread background pieces (next chunk's qkv, earlier
                        # chunks' projections) across the attention loop --
                        # emitted after the j-group so pair matmuls outrank
                        # fillers in scheduler priority
                        want = (nbg0 * (j + 1) + njr - 1) // njr
                        while emitted < want and bg:
                            kind, rr, p = bg.pop(0)
                            emitted += 1
                            if kind == "qkv":
                                emit_qkv_piece(rr, p)
                            else:
                                emit_proj_piece(rr, p)
                    # normalization per head, pure DVE: 1/l on the replicated
                    # l rows 64:128, then partition-shifted multiply into O_sb
                    for h in range(4):
                        pi, ch = divmod(h, 2)
                        rl = spool.tile([128, 512], bf, tag=f"rl{h}",
                                        name=f"rl{h}")
                        nc.vector.reciprocal(rl[64:128, :], O_ps[h][64:128, :])
                        nc.vector.tensor_mul(
                            O_sb[pi][64 * ch:64 * ch + 64,
                                     512 * R:512 * (R + 1)],
                            O_ps[h][0:64, :], rl[64:128, :])
                    # projection for this chunk rides a later chunk's loop
                    if interleave:
                        if NCH != 4 or R == NCH - 1:
                            bg.extend(("proj", R, p) for p in range(8))
                    else:
                        for p in range(8):
                            emit_proj_piece(R, p)

                # flush remaining pieces
                for kind, rr, p in bg:
                    if kind == "qkv":
                        emit_qkv_piece(rr, p)
                    else:
                        emit_proj_piece(rr, p)

    nc.compile()
    return nc


def make_core_inputs(x, w_qkv, b_qkv, w_proj, core, T=None):
    """Host-side shard/prep for one core. Returns the in_map dict."""
    if T is None:
        T = x.shape[1]
    b, g = divmod(core, 4)
    heads = [4 * g + i for i in range(HPC)]

    xT = np.ascontiguousarray(np.asarray(x[b], np.float32).T).astype(BF16)

    qcols = [w_qkv[:, h * HD:(h + 1) * HD] for h in heads]
    kcols = [w_qkv[:, C + h * HD:C + (h + 1) * HD] for h in heads]
    vcols = [w_qkv[:, 2 * C + h * HD:2 * C + (h + 1) * HD] for h in heads]
    wqk = np.concatenate(qcols + kcols, axis=1).astype(BF16)      # [C, 512]
    wv = np.concatenate(vcols, axis=1).astype(BF16)               # [C, 256]
    wp = np.concatenate([w_proj[h * HD:(h + 1) * HD, :] for h in heads],
                        axis=0).astype(BF16)                      # [256, C]

    bq = [b_qkv[h * HD:(h + 1) * HD] for h in heads]
    bk = [b_qkv[C + h * HD:C + (h + 1) * HD] for h in heads]
    bqk = np.concatenate(bq + bk).astype(np.float32).reshape(4, 128).T
    bqk = np.ascontiguousarray(bqk)                               # [128, 4]

    a = np.arange(128)
    tri = (a[:, None] <= a[None, :]).astype(BF16)                 # [128, 128]
    ones64 = np.ones((128, 64), dtype=BF16)

    return {
        "xT": xT, "wqkv": np.concatenate([wqk, wv], axis=1),
        "wp": wp, "bqk": bqk,
        "cst": np.concatenate([tri, ones64], axis=1),
    }


_compiled = {}


def _get_nc(T):
    if T not in _compiled:
        _compiled[T] = build_nc(T)
    return _compiled[T]


def kernel(x, w_qkv, b_qkv, w_proj, b_proj):
    from concourse.bass_utils import run_bass_kernel_spmd

    x = np.asarray(x, np.float32)
    w_qkv = np.asarray(w_qkv, np.float32)
    b_qkv = np.asarray(b_qkv, np.float32)
    w_proj = np.asarray(w_proj, np.float32)
    b_proj = np.asarray(b_proj, np.float32)
    T = x.shape[1]

    nc = _get_nc(T)
    in_maps = [make_core_inputs(x, w_qkv, b_qkv, w_proj, core, T)
               for core in range(NCORES)]
    res = run_bass_kernel_spmd(nc, in_maps, core_ids=list(range(NCORES)))

    y = np.zeros((B, T, C), np.float32)
    for core in range(NCORES):
        b = core // 4
        y[b] += res.results[core]["y"].astype(np.float32)
    y += b_proj[None, None, :] + (b_qkv[2 * C:3 * C] @ w_proj)[None, None, :]
    return y
